# revision 1
# baseline (speedup 1.0000x reference)
"""Trainium2 Bass kernel for InnerproductSimilarity — int8-quantized output.

Key ideas vs baseline:
  - Output stored as int8 q = round(126.5 * simi); host decodes
    out = 0.5 + q * (0.5/126.5).  Quant error 0.00198 abs (tolerance 2e-2
    rel of absmax ~0.82).  4x less store traffic than f32.
  - Matmul inputs bf16 (error ~0.004 abs on simi, still 4x under tolerance).
  - i-packed tiles: the 4*441=1764 query positions per core are tiled in
    14 chunks of 128 partitions spanning query boundaries -> all copies
    use (nearly) full 128 lanes.
  - Per i-tile the full 5n x 2205j = 11025 output columns stream through
    [128, 2048] f32 PSUM tiles (4 banks, bufs=2 = all 8 banks), so the
    PSUM->SBUF cast copies are 2048 wide (ACT ~1.85us, DVE ~2.26us per
    chunk), split ACT/DVE to balance.
  - Row-split matmul pairs: contraction K=64 uses only half the PE array;
    queries are duplicated into partitions 64-127 and support columns are
    split half/half so two concurrent matmuls (row groups 0-63 / 64-127)
    each stream half the columns.
  - Norms (L2 over c) computed on device: ACT Square, per-512-chunk column
    sums via one matmul per chunk against a shifted-ones stationary (packs
    norms^2 of chunk p into PSUM row p), Sqrt, DVE reciprocal, K=1
    broadcast matmuls, DVE multiply -> bf16 normalized operands.
    Query scale folds in the 126.5 quantization factor.
"""

import numpy as np

N_WAY = 5
K_SHOT = 5
C = 64
HW = 441
M_SUP = K_SHOT * HW      # 2205
Q = 32
N_CORES = 8
QPC = Q // N_CORES       # 4
GI = QPC * HW            # 1764 query positions per core
SF = N_WAY * M_SUP       # 11025 support cols (concat-j per i-tile)
SN2W = 5632              # per-half support layout width (5*1024 + 512)
QSCALE = 126.5

I_TILES = [(128 * t, min(128, GI - 128 * t)) for t in range((GI + 127) // 128)]


def _chunk_layout():
    """Per 2048-col psum chunk: (width, [(psum_off, w, half, sn2_col)])."""
    chunks = []
    for c in range(6):
        base = 2048 * c
        cw = 2048 if c < 5 else SF - 5 * 2048  # 785
        low = 1024 if c < 5 else 512
        pieces = []
        for h in (0, 1):
            u0, u1 = ((0, low) if h == 0 else (low, cw))
            s0 = 1024 * c if c < 5 else 5120
            u = u0
            while u < u1:
                cj = base + u
                nxt = min(u1, (u // 512 + 1) * 512, ((cj // M_SUP) + 1) * M_SUP - base)
                pieces.append((u, nxt - u, h, s0 + (u - u0)))
                u = nxt
        chunks.append((cw, pieces))
    return chunks


CHUNKS = _chunk_layout()

# store sub-ranges per i-tile: [(p0, cnt, q, i0)] split at query boundaries
def _store_ranges():
    out = []
    for (g0, im) in I_TILES:
        subs = []
        g = g0
        while g < g0 + im:
            q = g // HW
            g_end = min(g0 + im, (q + 1) * HW)
            subs.append((g - g0, g_end - g, q, g - q * HW))
            g = g_end
        out.append(subs)
    return out


STORES = _store_ranges()

_CACHE = {}

# bisect switches (set before _build)
VARIANT = {
    "fancy_store": True,
    "use_copy": True,
    "mm64": False,
    "mmf32": False,
    "hostnorm": True,
}


def _build(reps=1, mode="full"):
    key = ("nc", reps, mode, tuple(sorted(VARIANT.items())))
    if key in _CACHE:
        return _CACHE[key]
    import concourse.bacc as bacc
    import concourse.mybir as mybir
    import concourse.tile as tile

    nc = bacc.Bacc(
        "TRN2",
        target_bir_lowering=False,
        debug=False,
        enable_asserts=False,
        num_devices=N_CORES,
    )
    f32 = mybir.dt.float32
    bf16 = mybir.dt.bfloat16
    i8 = mybir.dt.int8
    AF = mybir.ActivationFunctionType

    indt = (f32 if VARIANT["mmf32"] else bf16) if VARIANT["hostnorm"] else f32
    q_in = nc.dram_tensor("q_in", [128, GI], indt, kind="ExternalInput").ap()
    s_in = nc.dram_tensor("s_in", [128, SN2W], indt, kind="ExternalInput").ap()
    sel_in = nc.dram_tensor("sel_in", [22, 64 * 22], f32, kind="ExternalInput").ap()
    selq_in = nc.dram_tensor("selq_in", [4, 512], f32, kind="ExternalInput").ap()
    out = nc.dram_tensor(
        "out", [QPC * N_WAY, HW, M_SUP], i8, kind="ExternalOutput"
    ).ap()

    with tile.TileContext(nc) as tc:
        with (
            tc.tile_pool(name="const", bufs=1) as const_pool,
            tc.tile_pool(name="io", bufs=1) as io_pool,
        ):
            # B128[:, c] = 1 iff c == 128; slice [128-p : 256-p] selects row p
            B128 = const_pool.tile([128, 256], f32)
            nc.vector.memset(B128, 0.0)
            nc.vector.memset(B128[:, 128:129], 1.0)
            # SEL[k, c] = 1 iff c in [64k, 64k+64): slice [64p : 64p+64] is a
            # rank-1 selector (row p of the rhs broadcast to all 64 out rows);
            # host-supplied (single-partition memsets are rejected by walrus)
            SEL = const_pool.tile([22, 64 * 22], f32)
            nc.sync.dma_start(out=SEL, in_=sel_in)
            SELQ = const_pool.tile([4, 512], f32)
            nc.sync.dma_start(out=SELQ, in_=selq_in)

            qsb = io_pool.tile([128, GI], f32)
            ssb = io_pool.tile([128, SN2W], f32)
            sq_q = io_pool.tile([64, GI], f32)
            sq2 = io_pool.tile([128, SN2W], f32)
            rq_row = io_pool.tile([4, HW], f32)
            rs_row = io_pool.tile([22, 512], f32)
            mmdt = f32 if VARIANT["mmf32"] else bf16
            qn2 = io_pool.tile([128, GI], mmdt)
            sn2 = io_pool.tile([128, SN2W], mmdt)

            for rep in range(reps):
                if VARIANT["hostnorm"]:
                    nc.sync.dma_start(out=qn2, in_=q_in)
                    nc.scalar.dma_start(out=sn2, in_=s_in)
                else:
                    nc.sync.dma_start(out=qsb, in_=q_in)
                    nc.scalar.dma_start(out=ssb, in_=s_in)

                with tc.tile_pool(name="psn", bufs=2, space="PSUM") as psn:
                    skip_norm = VARIANT["hostnorm"]
                    if not skip_norm:
                        # ---- query norms: rq = QSCALE/||q_col|| ----
                        nc.scalar.activation(sq_q, qsb[0:64, :], AF.Square)
                        psq = psn.tile([128, HW], f32, tag="psq")
                        for p in range(QPC):
                            nc.tensor.matmul(
                                psq[:, :],
                                B128[0:64, 128 - p : 256 - p],
                                sq_q[:, HW * p : HW * (p + 1)],
                                start=(p == 0),
                                stop=(p == QPC - 1),
                            )
                        nc.scalar.activation(
                            rq_row, psq[0:4, :], AF.Sqrt, scale=float(1.0 / QSCALE**2)
                        )
                        nc.vector.reciprocal(rq_row, rq_row)
                        # ---- support norms: rs = 1/||s_col|| (packed [22,512]) ----
                        for k in range(4):
                            sl = slice(1408 * k, 1408 * (k + 1))
                            nc.scalar.activation(sq2[:, sl], ssb[:, sl], AF.Square)
                        ps_ns = psn.tile([128, 512], f32, tag="ns")
                        for r in range(22):
                            h, p = (0, r) if r < 11 else (1, r - 11)
                            nc.tensor.matmul(
                                ps_ns[:, :],
                                B128[64 * h : 64 * h + 64, 128 - r : 256 - r],
                                sq2[64 * h : 64 * h + 64, 512 * p : 512 * (p + 1)],
                                start=(r == 0),
                                stop=(r == 21),
                            )
                        nc.scalar.activation(rs_row, ps_ns[0:22, :], AF.Sqrt)
                        nc.vector.reciprocal(rs_row, rs_row)
                        # ---- broadcast + scale -> bf16 operands ----
                        for q in range(QPC):
                            psbq = psn.tile([128, HW], f32, tag="bq")
                            nc.tensor.matmul(
                                psbq[:, :],
                                SELQ[0:4, 128 * q : 128 * (q + 1)],
                                rq_row[0:4, :],
                                start=True,
                                stop=True,
                            )
                            sl = slice(HW * q, HW * (q + 1))
                            nc.vector.tensor_mul(qn2[:, sl], qsb[:, sl], psbq[:, :])
                        for p in range(11):
                            psb = psn.tile([128, 512], f32, tag="bs")
                            nc.tensor.matmul(
                                psb[0:64, :],
                                SEL[0:22, 64 * p : 64 * (p + 1)],
                                rs_row[0:22, :],
                                start=True,
                                stop=True,
                            )
                            nc.tensor.matmul(
                                psb[64:128, :],
                                SEL[0:22, 64 * (11 + p) : 64 * (12 + p)],
                                rs_row[0:22, :],
                                start=True,
                                stop=True,
                            )
                            sl = slice(512 * p, 512 * (p + 1))
                            nc.vector.tensor_mul(sn2[:, sl], ssb[:, sl], psb[:, :])

                # ---- main loop ----
                with (
                    tc.tile_pool(name="psm", bufs=2, space="PSUM") as psm,
                    tc.tile_pool(name="outp", bufs=3) as out_pool,
                ):
                    fixed_osb = None
                    if mode == "dmaonly":
                        fixed_osb = out_pool.tile([128, SF], i8, tag="fixed")
                        nc.vector.memset(fixed_osb, 1)
                    for t, (g0, im) in enumerate(I_TILES):
                        if mode == "dmaonly":
                            osb = fixed_osb
                        else:
                            osb = out_pool.tile([128, SF], i8, tag="osb")
                            for c, (cw, pieces) in enumerate(CHUNKS):
                                ps = psm.tile([128, 2048], f32, tag="mm")
                                for (off, w, h, scol) in pieces:
                                    if VARIANT["mm64"]:
                                        h = 0
                                    nc.tensor.matmul(
                                        ps[:im, off : off + w],
                                        qn2[64 * h : 64 * h + 64, g0 : g0 + im],
                                        sn2[64 * h : 64 * h + 64, scol : scol + w],
                                        start=True,
                                        stop=True,
                                    )
                                dst = osb[:im, 2048 * c : 2048 * c + cw]
                                if c % 2 == 0:
                                    fn = AF.Copy if VARIANT["use_copy"] else AF.Identity
                                    nc.scalar.activation(dst, ps[:im, :cw], fn)
                                else:
                                    nc.vector.tensor_scalar_add(dst, ps[:im, :cw], 0.0)
                        if mode == "nodma" and rep != reps - 1:
                            continue
                        dma_eng = nc.sync if t % 2 == 0 else nc.scalar
                        for (p0, cnt, q, i0) in STORES[t]:
                            if VARIANT["fancy_store"]:
                                dma_eng.dma_start(
                                    out=out[
                                        N_WAY * q : N_WAY * (q + 1), i0 : i0 + cnt, :
                                    ].transpose([1, 0, 2]),
                                    in_=osb[p0 : p0 + cnt, :].rearrange(
                                        "p (n j) -> p n j", n=N_WAY
                                    ),
                                )
                            else:
                                for n in range(N_WAY):
                                    dma_eng.dma_start(
                                        out=out[N_WAY * q + n, i0 : i0 + cnt, :],
                                        in_=osb[
                                            p0 : p0 + cnt,
                                            M_SUP * n : M_SUP * (n + 1),
                                        ],
                                    )
    nc.compile()
    _CACHE[key] = nc
    return nc


def _get_runner(reps=1, mode="full"):
    """Cached jitted SPMD executor (same machinery as baseline kernel)."""
    key = ("runner", reps, mode, tuple(sorted(VARIANT.items())))
    if key in _CACHE:
        return _CACHE[key]
    import jax
    import jax.numpy as jnp
    from jax.experimental.shard_map import shard_map
    from jax.sharding import Mesh, NamedSharding, PartitionSpec

    import concourse.mybir as mybir
    from concourse import bass2jax

    nc = _build(reps, mode)
    bass2jax.install_neuronx_cc_hook()

    partition_name = nc.partition_id_tensor.name if nc.partition_id_tensor else None
    in_names, out_names, out_avals = [], [], []
    for alloc in nc.m.functions[0].allocations:
        if not isinstance(alloc, mybir.MemoryLocationSet):
            continue
        name = alloc.memorylocations[0].name
        if alloc.kind == "ExternalInput":
            if name == partition_name:
                continue
            in_names.append(name)
        elif alloc.kind == "ExternalOutput":
            out_names.append(name)
            out_avals.append(
                jax.core.ShapedArray(
                    tuple(alloc.tensor_shape), mybir.dt.np(alloc.dtype)
                )
            )
    n_params = len(in_names)
    n_outs = len(out_names)
    all_in_names = tuple(in_names) + tuple(out_names)
    if partition_name is not None:
        all_in_names = all_in_names + (partition_name,)

    def _body(*args):
        operands = list(args)
        if partition_name is not None:
            operands.append(bass2jax.partition_id_tensor())
        outs = bass2jax._bass_exec_p.bind(
            *operands,
            out_avals=tuple(out_avals),
            in_names=all_in_names,
            out_names=tuple(out_names),
            lowering_input_output_aliases=(),
            sim_require_finite=True,
            sim_require_nnan=True,
            nc=nc,
        )
        return tuple(outs)

    devices = jax.devices()[:N_CORES]
    assert len(devices) == N_CORES, f"need {N_CORES} cores, have {len(jax.devices())}"
    mesh = Mesh(np.asarray(devices), ("core",))
    in_specs = (PartitionSpec("core"),) * (n_params + n_outs)
    out_specs = (PartitionSpec("core"),) * n_outs
    donate = tuple(range(n_params, n_params + n_outs))
    sharded = jax.jit(
        shard_map(
            _body, mesh=mesh, in_specs=in_specs, out_specs=out_specs, check_rep=False
        ),
        donate_argnums=donate,
        keep_unused=True,
    )
    shard = NamedSharding(mesh, PartitionSpec("core"))
    zero_shapes = [(N_CORES * a.shape[0], *a.shape[1:]) for a in out_avals]
    zeros_fn = jax.jit(
        lambda: tuple(
            jnp.zeros(s, a.dtype) for s, a in zip(zero_shapes, out_avals)
        ),
        out_shardings=(shard,) * n_outs,
    )
    _CACHE[key] = (sharded, zeros_fn, in_names, out_names, shard)
    return _CACHE[key]


def _prep_inputs(support_xf, query_xf):
    """Host-side layout prep: channel-major, query-sharded, row-duplicated
    query + half-split interleaved support for row-group matmul pairs."""
    s_cm = np.ascontiguousarray(
        support_xf.reshape(N_WAY, K_SHOT, C, HW)
        .transpose(2, 0, 1, 3)
        .reshape(C, SF)
    ).astype(np.float32, copy=False)
    hostnorm = VARIANT["hostnorm"]
    if hostnorm:
        s_cm = s_cm / np.linalg.norm(s_cm, axis=0, keepdims=True)
    lo = np.concatenate(
        [s_cm[:, 2048 * c : 2048 * c + 1024] for c in range(5)]
        + [s_cm[:, 10240:10752]],
        axis=1,
    )
    hi = np.concatenate(
        [s_cm[:, 2048 * c + 1024 : 2048 * (c + 1)] for c in range(5)]
        + [s_cm[:, 10752:SF], np.ones((C, SN2W - 5120 - (SF - 10752)), np.float32)],
        axis=1,
    )
    s_half = np.concatenate([lo, hi], axis=0)  # [128, 5632]
    assert s_half.shape == (128, SN2W)

    q_all = query_xf.reshape(Q, C, HW)
    q_parts = []
    for k in range(N_CORES):
        q_cm = q_all[k * QPC : (k + 1) * QPC].transpose(1, 0, 2).reshape(C, GI)
        if hostnorm:
            q_cm = q_cm * (QSCALE / np.linalg.norm(q_cm, axis=0, keepdims=True))
        q_parts.append(np.concatenate([q_cm, q_cm], axis=0))  # [128, 1764]
    q_cat = np.concatenate(q_parts, axis=0)
    s_cat = np.concatenate([s_half] * N_CORES, axis=0)
    if hostnorm:
        import ml_dtypes

        dt = np.float32 if VARIANT["mmf32"] else ml_dtypes.bfloat16
        q_cat = q_cat.astype(dt)
        s_cat = s_cat.astype(dt)
    sel = np.zeros((22, 64 * 22), np.float32)
    for k in range(22):
        sel[k, 64 * k : 64 * (k + 1)] = 1.0
    selq = np.zeros((4, 512), np.float32)
    for k in range(QPC):
        selq[k, 128 * k : 128 * (k + 1)] = 1.0
    return {
        "q_in": np.ascontiguousarray(q_cat),
        "s_in": np.ascontiguousarray(s_cat),
        "sel_in": np.concatenate([sel] * N_CORES, axis=0),
        "selq_in": np.concatenate([selq] * N_CORES, axis=0),
    }


def kernel(support_xf, support_y, query_xf, query_y):
    import jax

    assert support_xf.shape == (1, N_WAY * K_SHOT, C, 21, 21)
    assert query_xf.shape == (1, Q, C, 21, 21)

    sharded, zeros_fn, in_names, out_names, shard = _get_runner()
    cat = _prep_inputs(support_xf, query_xf)
    args = [jax.device_put(cat[n], shard) for n in in_names]
    outs = sharded(*args, *zeros_fn())
    out_q = np.asarray(outs[0])  # [8*20, 441, 2205] int8, core-major
    out = out_q.astype(np.float32)
    out *= 0.5 / QSCALE
    out += 0.5
    return out.reshape(1, Q, N_WAY, HW, M_SUP)

